# revision 7
# baseline (speedup 1.0000x reference)
"""DiT self-attention Bass/Tile kernel for 8 Trainium2 NeuronCores.

Sharding: tensor-parallel over heads. Each of the 8 cores owns 2 of the 16
heads (a 128-wide slice of the hidden dim): it computes Q/K/V projections for
its heads over the full sequence, runs attention for its (batch, head) pairs,
and produces a partial output projection (row-sharded Wo). The host sums the
8 partials and adds the output bias.

Shapes are hardcoded for hidden_states [2, 2048, 1024], 16 heads, head dim 64.
"""
import numpy as np
import os
PHASES = int(os.environ.get('KPHASES', '3'))

import concourse.bass as bass
import concourse.tile as tile
from concourse import bacc, mybir
from concourse.bass_utils import run_bass_kernel_spmd

F32 = mybir.dt.float32
F32R = mybir.dt.float32r

B = 2
S = 2048
H = 1024
NS = B * S          # 4096 rows total
D = 128             # per-core hidden slice (2 heads)
HD = 64             # head dim
SB = 512            # sequence block for projections / attention q-blocks
NSB = NS // SB      # 8
NCHUNK = H // 128   # 8 contraction chunks for projections
NJ = NS // 128      # 32 key chunks globally
EXP = mybir.ActivationFunctionType.Exp

_CACHED = None


def _build():
    nc = bacc.Bacc("TRN2", target_bir_lowering=False, debug=False)

    hsT = nc.dram_tensor("hsT", [H, NS], F32R, kind="ExternalInput").ap()
    wq = nc.dram_tensor("wq", [H, D], F32R, kind="ExternalInput").ap()
    wk = nc.dram_tensor("wk", [H, D], F32R, kind="ExternalInput").ap()
    wv = nc.dram_tensor("wv", [H, D], F32R, kind="ExternalInput").ap()
    wo = nc.dram_tensor("wo", [D, H], F32R, kind="ExternalInput").ap()
    bq = nc.dram_tensor("bq", [D, 1], F32, kind="ExternalInput").ap()
    bk = nc.dram_tensor("bk", [D, 1], F32, kind="ExternalInput").ap()
    bv = nc.dram_tensor("bv", [D, 1], F32, kind="ExternalInput").ap()
    vones = nc.dram_tensor("vones", [128, 64], F32R, kind="ExternalInput").ap()
    ident = nc.dram_tensor("ident", [128, 128], F32R, kind="ExternalInput").ap()
    out = nc.dram_tensor("out", [NS, H], F32, kind="ExternalOutput").ap()

    hsT_r = hsT.rearrange("(c p) s -> p c s", p=128)

    with tile.TileContext(nc) as tc:
        with tc.tile_pool(name="singles", bufs=1) as sg:
            # persistent tensors
            qt = sg.tile([128, NS], F32R, tag="qt")
            kt = sg.tile([128, NS], F32R, tag="kt")
            va = sg.tile([128, NJ, 128], F32R, tag="va")
            vb = sg.tile([128, NJ, 128], F32R, tag="vb")
            ctxa = sg.tile([64, NS], F32R, tag="ctxa")
            ctxb = sg.tile([64, NS], F32R, tag="ctxb")
            cstack = sg.tile([128, NS], F32R, tag="cstack")
            wq_sb = sg.tile([128, NCHUNK, D], F32R, tag="wq")
            wk_sb = sg.tile([128, NCHUNK, D], F32R, tag="wk")
            wv_sb = sg.tile([128, NCHUNK, D], F32R, tag="wv")
            wo_sb = sg.tile([128, H], F32R, tag="wo")
            bq_sb = sg.tile([128, 1], F32, tag="bq")
            bk_sb = sg.tile([128, 1], F32, tag="bk")
            bv_sb = sg.tile([128, 1], F32, tag="bv")
            on_sb = sg.tile([128, 64], F32R, tag="vones")
            id_sb = sg.tile([128, 128], F32R, tag="ident")

            nc.sync.dma_start(out=wq_sb, in_=wq.rearrange("(c p) d -> p c d", p=128))
            nc.sync.dma_start(out=wk_sb, in_=wk.rearrange("(c p) d -> p c d", p=128))
            nc.sync.dma_start(out=wv_sb, in_=wv.rearrange("(c p) d -> p c d", p=128))
            nc.sync.dma_start(out=wo_sb, in_=wo)
            nc.sync.dma_start(out=bq_sb, in_=bq)
            nc.sync.dma_start(out=bk_sb, in_=bk)
            nc.sync.dma_start(out=bv_sb, in_=bv)
            nc.sync.dma_start(out=on_sb, in_=vones)
            nc.sync.dma_start(out=id_sb, in_=ident)

            # ---------------- phase 1: projections + V transpose ------------
            psu = sg.enter_psum = tc.tile_pool(name="psu", bufs=1, space="PSUM")
            ps1 = ps2 = ps3 = psu.__enter__()
            with tc.tile_pool(name="p1sb", bufs=1) as p1sb:
                vt = p1sb.tile([128, NS], F32R, tag="vt")
                for sb in range(NSB):
                    hs = p1sb.tile([128, NCHUNK, SB], F32R, tag="hs", bufs=2)
                    for cth in range(NCHUNK):
                        nc.sync.dma_start(
                            out=hs[:, cth, :],
                            in_=hsT_r[:, cth, sb * SB:(sb + 1) * SB])
                    for wsb, bsb, dest in ((wq_sb, bq_sb, qt),
                                           (wk_sb, bk_sb, kt),
                                           (wv_sb, bv_sb, vt)):
                        pp = ps1.tile([128, SB], F32, tag="misc", bufs=2)
                        for cth in range(NCHUNK):
                            nc.tensor.matmul(pp, lhsT=wsb[:, cth, :],
                                             rhs=hs[:, cth, :],
                                             start=(cth == 0),
                                             stop=(cth == NCHUNK - 1))
                        nc.vector.tensor_scalar_add(
                            dest[:, sb * SB:(sb + 1) * SB], pp, bsb)
                    # transpose V for the 4 key-chunks this s-block covers
                    for j in range(sb * 4, sb * 4 + 4):
                        tpf = ps1.tile([128, SB], F32R, tag="misc", bufs=2,
                                       name=f"tr{j}")
                        tp = tpf[:, 0:128]
                        nc.tensor.transpose(tp, vt[:, j * 128:(j + 1) * 128], id_sb)
                        nc.vector.tensor_copy(va[:, j, 0:64], tp[:, 0:64])
                        nc.vector.tensor_copy(vb[:, j, 0:64], tp[:, 64:128])
                        nc.vector.tensor_copy(va[:, j, 64:128], on_sb)
                        nc.vector.tensor_copy(vb[:, j, 64:128], on_sb)

            # ---------------- phase 2+3: attention, assemble, out-proj ------
            with tc.tile_pool(name="p2sb", bufs=1) as p2sb:
                for b in range(B if PHASES >= 2 else 0):
                    bcol = b * S
                    for hh in range(2):
                        part = slice(hh * 64, hh * 64 + 64)
                        vsel = va if hh == 0 else vb
                        ctxd = ctxa if hh == 0 else ctxb
                        QW = 1024
                        for qb in range(S // QW):
                            qcols = slice(bcol + qb * QW, bcol + (qb + 1) * QW)
                            cp = ps2.tile([128, QW], F32, tag="ctx", bufs=1)
                            for cc in range(16):
                                kcols = slice(bcol + cc * 128, bcol + (cc + 1) * 128)
                                sp = ps2.tile([128, QW], F32, tag="s", bufs=2)
                                for qh in range(QW // SB):
                                    nc.tensor.matmul(
                                        sp[:, qh * SB:(qh + 1) * SB],
                                        lhsT=kt[part, kcols],
                                        rhs=qt[part, bcol + qb * QW + qh * SB:
                                               bcol + qb * QW + (qh + 1) * SB],
                                        start=True, stop=True)
                                et = p2sb.tile([128, QW], F32R, tag="e", bufs=4)
                                nc.scalar.activation(out=et, in_=sp, func=EXP,
                                                     scale=0.125)
                                for qh in range(QW // SB):
                                    nc.tensor.matmul(
                                        cp[:, qh * SB:(qh + 1) * SB],
                                        lhsT=vsel[:, b * 16 + cc, :],
                                        rhs=et[:, qh * SB:(qh + 1) * SB],
                                        start=(cc == 0), stop=(cc == 15))
                            # rows 0:64 = ctx^T, rows 64:128 = sumexp replicated
                            rc = p2sb.tile([128, QW], F32, tag="rc", bufs=2)
                            nc.vector.reciprocal(rc[64:128, :], cp[64:128, :])
                            rlo = p2sb.tile([64, QW], F32, tag="rlo", bufs=2)
                            nc.sync.dma_start(out=rlo, in_=rc[64:128, :])
                            nc.vector.tensor_mul(ctxd[:, qcols], cp[0:64, :], rlo)
                    # assemble this batch's ctx columns into cstack
                    for ci in range(4):
                        bc = slice(bcol + ci * SB, bcol + (ci + 1) * SB)
                        nc.sync.dma_start(out=cstack[0:64, bc], in_=ctxa[:, bc])
                        nc.sync.dma_start(out=cstack[64:128, bc], in_=ctxb[:, bc])
                    # output projection for this batch's rows
                    for qc in range(b * 16, (b + 1) * 16 if PHASES >= 3 else b * 16):
                        for nb in range(2):
                            op = ps3.tile([128, SB], F32, tag="misc", bufs=2)
                            nc.tensor.matmul(
                                op, lhsT=cstack[:, qc * 128:(qc + 1) * 128],
                                rhs=wo_sb[:, nb * SB:(nb + 1) * SB],
                                start=True, stop=True)
                            ot = p2sb.tile([128, SB], F32, tag="ot", bufs=3)
                            nc.vector.tensor_copy(ot, op)
                            nc.sync.dma_start(
                                out=out[qc * 128:(qc + 1) * 128,
                                        nb * SB:(nb + 1) * SB],
                                in_=ot)

            psu.__exit__(None, None, None)
    nc.compile()
    return nc


def _get_program():
    global _CACHED
    if _CACHED is None:
        _CACHED = _build()
    return _CACHED


def kernel(hidden_states, Wq, bq, Wk, bk, Wv, bv, Wo, bo):
    nc = _get_program()
    hs = np.asarray(hidden_states, dtype=np.float32).reshape(NS, H)
    hsT = np.ascontiguousarray(hs.T)
    Wq = np.asarray(Wq, dtype=np.float32)
    Wk = np.asarray(Wk, dtype=np.float32)
    Wv = np.asarray(Wv, dtype=np.float32)
    Wo = np.asarray(Wo, dtype=np.float32)
    vones = np.ones((128, 64), dtype=np.float32)
    ident = np.eye(128, dtype=np.float32)

    in_maps = []
    for c in range(8):
        r = slice(D * c, D * (c + 1))
        in_maps.append({
            "hsT": hsT,
            "wq": np.ascontiguousarray(Wq[r].T),
            "wk": np.ascontiguousarray(Wk[r].T),
            "wv": np.ascontiguousarray(Wv[r].T),
            "wo": np.ascontiguousarray(Wo[:, r].T),
            "bq": np.asarray(bq, dtype=np.float32)[r].reshape(D, 1),
            "bk": np.asarray(bk, dtype=np.float32)[r].reshape(D, 1),
            "bv": np.asarray(bv, dtype=np.float32)[r].reshape(D, 1),
            "vones": vones,
            "ident": ident,
        })

    res = run_bass_kernel_spmd(nc, in_maps, list(range(8)))
    acc = np.zeros((NS, H), dtype=np.float64)
    for r_ in res.results:
        acc += r_["out"]
    acc += np.asarray(bo, dtype=np.float64)
    return acc.reshape(B, S, H).astype(np.float32)
